# revision 9
# baseline (speedup 1.0000x reference)
"""Trainium2 Bass kernel: GroupNorm(32) + single-head self-attention block + residual.

fp8 (e4m3) DoubleRow edition with fused score projection. Per image:
    h  = group_norm(x)  (fp32 stats; normalized output quantized to fp8)
    sT[m, n] = h_m^T G h_n,  G = wk^T wq   (one Z = G h projection replaces
        separate Q and K projections; the per-n bias terms cancel in softmax,
        exact when bq == 0 -- the general-bias fallback kernel keeps Q/K)
    p = exp(sT/sqrt(C) - 2)  (shift keeps exp < 240 = e4m3 max; softmax is
        shift-invariant since denom uses the same shifted values)   fp8
    v  = h^T wv^T   [HW, C] fp8 (bv folded into boP = bo + wo@bv)
    aT[c, n] = (sum_m v[m,c] p[m,n]) / denom[n]    fp8
    y  = wo @ aT + boP + x                          [C, HW] fp32

All heavy matmuls run fp8e4 with MatmulPerfMode.DoubleRow: operands carry the
contraction split [128 partitions, 2 k-tiles, free], one instruction contracts
256 x 512 output columns (measured ~2.6x fp32r per MAC). Weights are paired +
quantized on host and DMA'd as fp8 (no on-chip conversion).

Engine budget per image (approx): PE 33us, DVE 24us, ACT 21us, GpSimd 15us.
Evacs: Z/V/A on DVE, exp on ACT, OUT on ACT-Copy + gpsimd residual add,
recip via reciprocal_approx_fast, GN affine split gpsimd/DVE.

Sharding: data-parallel over batch; 8 cores x 4 images each.
"""

import math
import os

import numpy as np
import ml_dtypes

import concourse.bass as bass
import concourse.tile as tile
from concourse import bacc, mybir
from concourse.bass_utils import run_bass_kernel_spmd

N_CORES = 8
B, C, H, W = 32, 512, 32, 32
HW = H * W                      # 1024 tokens
BL = B // N_CORES               # 4 images per core
NGRP = 32                       # groupnorm groups
GS = C // NGRP                  # 16 channels per group
EPS = 1e-5
P = 128
NT = C // P                     # 4 channel partition-tiles
KP = NT // 2                    # 2 channel k-tile pairs (DoubleRow)
MT = HW // P                    # 8 token partition-tiles
MP = MT // 2                    # 4 token k-tile pairs
FCH = 512                       # output free-dim chunk (one PSUM bank fp32)
NCH = HW // FCH                 # 2 free chunks per 1024
F32 = mybir.dt.float32
F8 = mybir.dt.float8e4
DR = mybir.MatmulPerfMode.DoubleRow
SCALE = 1.0 / math.sqrt(C)
ESHIFT = -2.0                   # exp shift: keeps exp(s) under e4m3 max 240

ACT_EXP = mybir.ActivationFunctionType.Exp
ACT_LN = mybir.ActivationFunctionType.Ln
ACT_COPY = mybir.ActivationFunctionType.Copy
OP_ADD = mybir.AluOpType.add
OP_MULT = mybir.AluOpType.mult

LAST_EXEC_NS = None
_CACHED = {}


def _build_nc(fused):
    from contextlib import ExitStack

    nc = bacc.Bacc("TRN2", target_bir_lowering=False, debug=False)

    x_d = nc.dram_tensor("x", [BL, C, HW], F32, kind="ExternalInput").ap()
    # paired fp8 weights: [p, k, i, o] = w.T[(2k+i)*128+p, o]
    # fused mode: the wq8 slot carries G^T pairs for Z = G h (G = wk^T wq)
    wq8_d = nc.dram_tensor("wq8", [P, KP, 2, C], F8, kind="ExternalInput").ap()
    wk8_d = nc.dram_tensor("wk8", [P, KP, 2, C], F8, kind="ExternalInput").ap()
    wv8_d = nc.dram_tensor("wv8", [P, KP, 2, C], F8, kind="ExternalInput").ap()
    wo8_d = nc.dram_tensor("wo8", [P, KP, 2, C], F8, kind="ExternalInput").ap()
    ones8_d = nc.dram_tensor("ones8", [P, 2, P], F8, kind="ExternalInput").ap()
    bq_d = nc.dram_tensor("bq", [C], F32, kind="ExternalInput").ap()
    bk_d = nc.dram_tensor("bk", [C], F32, kind="ExternalInput").ap()
    boP_d = nc.dram_tensor("boP", [C], F32, kind="ExternalInput").ap()
    gw_d = nc.dram_tensor("gw", [C], F32, kind="ExternalInput").ap()
    gb_d = nc.dram_tensor("gb", [C], F32, kind="ExternalInput").ap()
    gm_d = nc.dram_tensor("gm", [P, NT, NGRP], F32, kind="ExternalInput").ap()
    gmt_d = nc.dram_tensor("gmt", [NGRP, NT, P], F32, kind="ExternalInput").ap()
    y_d = nc.dram_tensor("y", [BL, C, HW], F32, kind="ExternalOutput").ap()

    x_r = x_d.rearrange("b (t p) n -> b t p n", p=P)
    y_r = y_d.rearrange("b (t p) n -> b t p n", p=P)

    ib = lambda k, d: int(os.environ.get(k, d))  # buf-count knobs for tuning
    with tile.TileContext(nc) as tc, ExitStack() as ctx:
        pool = lambda name, bufs, space="SBUF": ctx.enter_context(
            tc.tile_pool(name=name, bufs=bufs, space=space)
        )
        p_const = pool("const", 1)
        p_x = pool("x", ib("BUF_X", 12))
        p_X = pool("X", ib("BUF_XN", 2))
        p_z = pool("z", 2)
        p_kt = pool("kt", 2)
        p_v = pool("v", 2)
        p_e = pool("exp", ib("BUF_EXP", 8))
        p_a = pool("a", 2)
        p_recip = pool("recip", 2)
        p_out = pool("out", ib("BUF_OUT", 4))
        p_small = pool("small", 4)
        psum = pool("psum", ib("BUF_PSUM", 8), space="PSUM")

        def ps_tile(name, parts=P, free=FCH):
            return psum.tile([parts, free], F32, tag="u", name=name)

        # ---- x loads; image 0 uses finer chunks so GN stats start sooner ----
        def emit_x(b, chunks=NCH):
            xt = []
            fch = HW // chunks
            for t in range(NT):
                xtile = p_x.tile([P, HW], F32, tag="x", name=f"x_{b}_{t}")
                for i in range(chunks):
                    nc.sync.dma_start(
                        out=xtile[:, i * fch : (i + 1) * fch],
                        in_=x_r[b, t][:, i * fch : (i + 1) * fch],
                    )
                xt.append(xtile)
            return xt

        xt0 = emit_x(0, chunks=4)

        # ---- small constants ----
        def load_cols(dram, tag):
            t = p_const.tile([P, NT], F32, tag=tag)
            nc.sync.dma_start(out=t[:], in_=dram.rearrange("(t p) -> p t", p=P))
            return t

        bq_sb = load_cols(bq_d, "bq")
        bk_sb = load_cols(bk_d, "bk")
        boP_sb = load_cols(boP_d, "boP")
        gw_sb = load_cols(gw_d, "gw")
        gb_sb = load_cols(gb_d, "gb")

        gm_sb = p_const.tile([P, NT, NGRP], F32, tag="gm")
        nc.sync.dma_start(out=gm_sb[:], in_=gm_d)
        gmt_sb = p_const.tile([NGRP, NT, P], F32, tag="gmt")
        nc.sync.dma_start(out=gmt_sb[:], in_=gmt_d)
        eps_sb = p_const.tile([P, 1], F32, tag="eps")
        nc.vector.memset(eps_sb[:], EPS)
        esh_sb = p_const.tile([P, 1], F32, tag="esh")
        nc.vector.memset(esh_sb[:], ESHIFT)

        # ---- fp8 weights: direct DMA, no staging/conversion ----
        def load_w8(dram, tag):
            t = p_const.tile([P, KP, 2, C], F8, tag=tag)
            nc.sync.dma_start(out=t[:], in_=dram)
            return t

        wq8 = load_w8(wq8_d, "wq8")          # Z weights (G pairs) when fused
        wk8 = None if fused else load_w8(wk8_d, "wk8")
        wv8 = load_w8(wv8_d, "wv8")
        wo8 = load_w8(wo8_d, "wo8")
        ones8 = p_const.tile([P, 2, P], F8, tag="ones8")
        nc.sync.dma_start(out=ones8[:], in_=ones8_d)

        def emit_gn_stats(b, xt, chunks=NCH):
            """DVE-only per-tile stats: stat2 = [mean, var + mean^2] per channel."""
            fch = HW // chunks
            stat2s = []
            for t in range(NT):
                st = p_small.tile([P, chunks, 6], F32, tag=f"bnst{chunks}")
                for i in range(chunks):
                    nc.vector.bn_stats(
                        out=st[:, i, :], in_=xt[t][:, i * fch : (i + 1) * fch]
                    )
                mv = p_small.tile([P, 2], F32, tag="bnmv")
                nc.vector.bn_aggr(out=mv[:], in_=st[:])
                stat2 = p_small.tile([P, 2], F32, tag="stat2", name=f"stat2_{b}_{t}")
                nc.vector.tensor_copy(out=stat2[:, 0:1], in_=mv[:, 0:1])
                m2 = p_small.tile([P, 1], F32, tag="m2")
                nc.vector.tensor_mul(m2[:], mv[:, 0:1], mv[:, 0:1])
                nc.vector.tensor_add(stat2[:, 1:2], mv[:, 1:2], m2[:])
                stat2s.append(stat2)
            return xt, stat2s

        def emit_gn_reduce(b, state):
            """Group-reduce via PE; rstd = rsqrt(var+eps)."""
            xt, stat2s = state
            psg = ps_tile(f"psg_{b}", parts=NGRP, free=2)
            for t in range(NT):
                nc.tensor.matmul(
                    psg[:], gm_sb[:, t, :], stat2s[t][:],
                    start=(t == 0), stop=(t == NT - 1),
                )
            # gmr: [32 groups, (mean, rstd)]
            gmr = p_small.tile([NGRP, 2], F32, tag="gmr")
            nc.vector.tensor_scalar_mul(gmr[:, 0:1], psg[:, 0:1], 1.0 / GS)
            e2g = p_small.tile([NGRP, 1], F32, tag="e2g")
            nc.vector.tensor_scalar_mul(e2g[:], psg[:, 1:2], 1.0 / GS)
            m2g = p_small.tile([NGRP, 1], F32, tag="m2g")
            nc.vector.tensor_mul(m2g[:], gmr[:, 0:1], gmr[:, 0:1])
            varg = p_small.tile([NGRP, 1], F32, tag="varg")
            nc.vector.tensor_sub(varg[:], e2g[:], m2g[:])
            if fused:
                # DVE-only rsqrt: group var is ~1 for standardized x (host
                # checks and falls back otherwise), so Newton from y0=1
                # converges quadratically; 3 iterations reach <1e-6 for
                # var in [0.5, 2]. Avoids ACT act-table loads entirely.
                ve = p_small.tile([NGRP, 1], F32, tag="ve")
                nc.vector.tensor_scalar_add(ve[:], varg[:], EPS)
                y = p_small.tile([NGRP, 1], F32, tag="nwy")
                nc.vector.tensor_scalar(
                    out=y[:], in0=ve[:], scalar1=-0.5, scalar2=1.5,
                    op0=OP_MULT, op1=OP_ADD,
                )
                for _ in range(2):
                    t1 = p_small.tile([NGRP, 1], F32, tag="nwt")
                    nc.vector.tensor_mul(t1[:], ve[:], y[:])
                    nc.vector.tensor_mul(t1[:], t1[:], y[:])
                    nc.vector.tensor_scalar(
                        out=t1[:], in0=t1[:], scalar1=-0.5, scalar2=1.5,
                        op0=OP_MULT, op1=OP_ADD,
                    )
                    nc.vector.tensor_mul(y[:], y[:], t1[:])
                nc.vector.tensor_copy(out=gmr[:, 1:2], in_=y[:])
            else:
                lng = p_small.tile([NGRP, 1], F32, tag="lng")
                nc.scalar.activation(
                    out=lng[:], in_=varg[:], func=ACT_LN, bias=eps_sb[0:NGRP, :]
                )
                nc.scalar.activation(
                    out=gmr[:, 1:2], in_=lng[:], func=ACT_EXP, scale=-0.5
                )
            return xt, gmr

        def emit_gn_norm(b, state):
            """Broadcast group stats to channels, apply affine -> fp8 pairs.
            Affine split across gpsimd and DVE to halve its latency."""
            xt, gmr = state
            X8 = p_X.tile([P, KP, 2, HW], F8, tag="X", name=f"X_{b}")
            for t in range(NT):
                psb = ps_tile(f"psb_{b}_{t}", free=2)
                nc.tensor.matmul(psb[:], gmt_sb[:, t, :], gmr[:], start=True, stop=True)
                acol = p_small.tile([P, 1], F32, tag="acol")
                nc.vector.tensor_mul(acol[:], psb[:, 1:2], gw_sb[:, t : t + 1])
                tmb = p_small.tile([P, 1], F32, tag="tmb")
                nc.vector.tensor_mul(tmb[:], psb[:, 0:1], acol[:])
                bcol = p_small.tile([P, 1], F32, tag="bcol")
                nc.vector.tensor_sub(bcol[:], gb_sb[:, t : t + 1], tmb[:])
                eng = nc.gpsimd if t < 3 else nc.vector
                eng.tensor_scalar(
                    out=X8[:, t // 2, t % 2, :], in0=xt[t][:],
                    scalar1=acol[:], scalar2=bcol[:], op0=OP_MULT, op1=OP_ADD,
                )
            return xt, X8

        xt_pre = {1: emit_x(1)} if BL > 1 else {}
        gn_state = emit_gn_norm(0, emit_gn_reduce(0, emit_gn_stats(0, xt0, chunks=4)))

        # ---- per-image heavy phases ----
        for b in range(BL):
            xt, X8 = gn_state
            # prefetch x two images ahead so bn_stats(b+1) never waits on DMA
            if b + 2 < BL:
                xt_pre[b + 2] = emit_x(b + 2)
            xt_next = xt_pre.get(b + 1)

            # channel-major projection [C, HW] with DVE evac (+ optional bias)
            def proj_cm(w8, bias_sb, out_pool, bname):
                dst = out_pool.tile(
                    [P, KP, 2, HW], F8, tag=bname, name=f"{bname}_{b}"
                )
                for ot in range(NT):
                    for nch in range(NCH):
                        ps = ps_tile(f"ps_{bname}_{b}_{ot}_{nch}")
                        for k in range(KP):
                            nc.tensor.matmul(
                                ps[:],
                                w8[:, k, :, ot * P : (ot + 1) * P],
                                X8[:, k, :, nch * FCH : (nch + 1) * FCH],
                                start=(k == 0),
                                stop=(k == KP - 1),
                                perf_mode=DR,
                            )
                        dslice = dst[:, ot // 2, ot % 2, nch * FCH : (nch + 1) * FCH]
                        if bias_sb is None:
                            nc.vector.tensor_copy(out=dslice, in_=ps[:])
                        else:
                            nc.vector.tensor_scalar(
                                out=dslice, in0=ps[:],
                                scalar1=bias_sb[:, ot : ot + 1], scalar2=None,
                                op0=OP_ADD,
                            )
                return dst

            if fused:
                Z8 = proj_cm(wq8, None, p_z, "z")
                s_stat, s_mov = X8, Z8
            else:
                QT8 = proj_cm(wq8, bq_sb, p_z, "q")
                KT8 = proj_cm(wk8, bk_sb, p_kt, "k")
                s_stat, s_mov = KT8, QT8

            # scores S^T[m, n] -> exp (fp8, shifted); per-m-pair tiles so PV
            # can start before the whole phase has evacuated
            E8 = [
                p_e.tile([P, 2, HW], F8, tag="exp", name=f"e_{b}_{j}")
                for j in range(MP)
            ]
            for mt in range(MT):
                for nch in range(NCH):
                    psS = ps_tile(f"ps_s_{b}_{mt}_{nch}")
                    for k in range(KP):
                        nc.tensor.matmul(
                            psS[:],
                            s_stat[:, k, :, mt * P : (mt + 1) * P],
                            s_mov[:, k, :, nch * FCH : (nch + 1) * FCH],
                            start=(k == 0),
                            stop=(k == KP - 1),
                            perf_mode=DR,
                        )
                    nc.scalar.activation(
                        out=E8[mt // 2][:, mt % 2, nch * FCH : (nch + 1) * FCH],
                        in_=psS[:], func=ACT_EXP, scale=SCALE, bias=esh_sb[:],
                    )

            # GN(b+1): stats in the S-phase shadow; the affine (gpsimd/DVE)
            # runs during V+PV so X8(b+1) is ready before the next Z-proj
            if xt_next is not None:
                gn_state = emit_gn_norm(
                    b + 1, emit_gn_reduce(b + 1, emit_gn_stats(b + 1, xt_next))
                )

            # V projection (token-major pairs); emitted after S so the PE
            # stays busy while ACT drains the exp backlog PV depends on
            V8 = p_v.tile([P, MP, 2, C], F8, tag="v", name=f"v_{b}")
            for mt in range(MT):
                ps = ps_tile(f"ps_v_{b}_{mt}")
                for k in range(KP):
                    nc.tensor.matmul(
                        ps[:],
                        X8[:, k, :, mt * P : (mt + 1) * P],
                        wv8[:, k, :, :],
                        start=(k == 0),
                        stop=(k == KP - 1),
                        perf_mode=DR,
                    )
                nc.vector.tensor_copy(out=V8[:, mt // 2, mt % 2, :], in_=ps[:])

            # A^T[c, n] accumulated over m-pairs, normalized by 1/denom.
            # colsum emitted after PV c2=0 so the PE never stalls on ACT.
            recip = p_recip.tile([P, HW], F32, tag="recip", name=f"recip_{b}")
            A8 = p_a.tile([P, KP, 2, HW], F8, tag="a", name=f"a_{b}")
            for c2 in range(NT):
                psA = []
                for nch in range(NCH):
                    ps_at = ps_tile(f"ps_a_{b}_{c2}_{nch}")
                    for j in range(MP):
                        nc.tensor.matmul(
                            ps_at[:],
                            V8[:, j, :, c2 * P : (c2 + 1) * P],
                            E8[j][:, :, nch * FCH : (nch + 1) * FCH],
                            start=(j == 0),
                            stop=(j == MP - 1),
                            perf_mode=DR,
                        )
                    psA.append(ps_at)
                if c2 == 0:
                    for nch in range(NCH):
                        psc_t = ps_tile(f"psc_{b}_{nch}")
                        for j in range(MP):
                            nc.tensor.matmul(
                                psc_t[:],
                                ones8[:],
                                E8[j][:, :, nch * FCH : (nch + 1) * FCH],
                                start=(j == 0),
                                stop=(j == MP - 1),
                                perf_mode=DR,
                            )
                        nc.vector.reciprocal_approx_fast(
                            out=recip[:, nch * FCH : (nch + 1) * FCH], in_=psc_t[:]
                        )
                for nch in range(NCH):
                    nc.vector.tensor_mul(
                        A8[:, c2 // 2, c2 % 2, nch * FCH : (nch + 1) * FCH],
                        psA[nch][:],
                        recip[:, nch * FCH : (nch + 1) * FCH],
                    )

            # output projection + bias + residual (fp32 out)
            for co in range(NT):
                for nch in range(NCH):
                    ps = ps_tile(f"ps_o_{b}_{co}_{nch}")
                    for k in range(KP):
                        nc.tensor.matmul(
                            ps[:],
                            wo8[:, k, :, co * P : (co + 1) * P],
                            A8[:, k, :, nch * FCH : (nch + 1) * FCH],
                            start=(k == 0),
                            stop=(k == KP - 1),
                            perf_mode=DR,
                        )
                    ot = p_out.tile([P, FCH], F32, tag="out", name=f"o_{b}_{co}_{nch}")
                    if fused:
                        # boP == 0 here: ACT Copy evac, residual add on gpsimd
                        tmp = p_out.tile(
                            [P, FCH], F32, tag="tmp", name=f"t_{b}_{co}_{nch}"
                        )
                        nc.scalar.activation(
                            out=tmp[:], in_=ps[:], func=ACT_COPY, bias=0.0
                        )
                        nc.gpsimd.tensor_add(
                            ot[:], tmp[:], xt[co][:, nch * FCH : (nch + 1) * FCH]
                        )
                    else:
                        nc.vector.scalar_tensor_tensor(
                            out=ot[:], in0=ps[:], scalar=boP_sb[:, co : co + 1],
                            in1=xt[co][:, nch * FCH : (nch + 1) * FCH],
                            op0=OP_ADD, op1=OP_ADD,
                        )
                    for h in range(2):
                        nc.sync.dma_start(
                            out=y_r[b, co][
                                :, nch * FCH + h * (FCH // 2) : nch * FCH + (h + 1) * (FCH // 2)
                            ],
                            in_=ot[:, h * (FCH // 2) : (h + 1) * (FCH // 2)],
                        )

    nc.compile()
    return nc


def _host_inputs(fused, x, gn_scale, gn_bias, wq, bq, wk, bk, wv, bv, wo, bo):
    f = lambda a: np.ascontiguousarray(np.asarray(a, dtype=np.float32))
    x = f(x).reshape(B, C, HW)
    boP = f(bo) + f(wo) @ f(bv)

    def pair8(wT):
        # [p, k, i, o] = wT[(2k+i)*128+p, o], quantized to e4m3
        t = np.ascontiguousarray(wT).reshape(KP, 2, P, C).transpose(2, 0, 1, 3)
        return np.ascontiguousarray(
            np.clip(t, -240, 240).astype(ml_dtypes.float8_e4m3)
        )

    if fused:
        G = f(wk).T @ f(wq)          # S^T[m,n] = h_m^T G h_n
        wq8 = pair8(G.T)             # stationary pairs of G: [p,k,i,o]=G[o,c]
        wk8 = np.zeros((P, KP, 2, C), ml_dtypes.float8_e4m3)
    else:
        wq8 = pair8(f(wq).T)
        wk8 = pair8(f(wk).T)

    gm = np.zeros((P, NT, NGRP), np.float32)
    gmt = np.zeros((NGRP, NT, P), np.float32)
    for t in range(NT):
        for p in range(P):
            g = (t * P + p) // GS
            gm[p, t, g] = 1.0
            gmt[g, t, p] = 1.0
    ones8 = np.ones((P, 2, P), ml_dtypes.float8_e4m3)

    shared = {
        "wq8": wq8,
        "wk8": wk8,
        "wv8": pair8(f(wv).T),
        "wo8": pair8(f(wo).T),
        "bq": f(bq), "bk": f(bk), "boP": boP,
        "gw": f(gn_scale), "gb": f(gn_bias),
        "gm": gm, "gmt": gmt, "ones8": ones8,
    }
    in_maps = []
    for i in range(N_CORES):
        m = dict(shared)
        m["x"] = np.ascontiguousarray(x[i * BL : (i + 1) * BL])
        in_maps.append(m)
    return in_maps


def kernel(x, gn_scale, gn_bias, wq, bq, wk, bk, wv, bv, wo, bo):
    global LAST_EXEC_NS
    assert x.shape == (B, C, H, W)
    boP = np.asarray(bo, np.float32) + np.asarray(wo, np.float32) @ np.asarray(
        bv, np.float32
    )
    # Z-fusion drops bq into softmax-invariant terms and needs boP == 0 for
    # the bias-free OUT evac; other biases fold exactly in both modes. The
    # fused GN rstd uses Newton-from-1 which needs roughly unit group
    # variance, so oddly-scaled x also routes to the general kernel.
    xs = np.asarray(x, np.float32).ravel()[::1031][:8192]
    fused = (
        not (np.any(np.asarray(bq)) or np.any(boP))
        and 0.5 < float(xs.var()) < 2.0
    )
    if fused not in _CACHED:
        _CACHED[fused] = _build_nc(fused)
    in_maps = _host_inputs(
        fused, x, gn_scale, gn_bias, wq, bq, wk, bk, wv, bv, wo, bo
    )
    trace = os.environ.get("ATT_TRACE", "0") == "1"
    if not trace:
        # the NTFF trace path needs antenv.axon_hooks (shimmed only by our
        # test harness); make sure a stray BASS_TRACE can't drag us into it
        os.environ["BASS_NEVER_TRACE"] = "1"
    else:
        os.environ.pop("BASS_NEVER_TRACE", None)
    kwargs = {}
    tdir = os.environ.get("ATT_TRACE_DIR")
    if tdir:
        kwargs["tmpdir"] = tdir
    res = run_bass_kernel_spmd(
        _CACHED[fused], in_maps, core_ids=list(range(N_CORES)), trace=trace, **kwargs
    )
    LAST_EXEC_NS = res.exec_time_ns
    y = np.concatenate([res.results[i]["y"] for i in range(N_CORES)], axis=0)
    return y.reshape(B, C, H, W).astype(np.float32)


# revision 10
# speedup vs baseline: 1.2157x; 1.2157x over previous
"""Trainium2 Bass kernel: GroupNorm(32) + single-head self-attention block + residual.

fp8 (e4m3) DoubleRow edition with fused score projection. Per image:
    h  = group_norm(x)  (fp32 stats; normalized output quantized to fp8)
    sT[m, n] = h_m^T G h_n,  G = wk^T wq   (one Z = G h projection replaces
        separate Q and K projections; the per-n bias terms cancel in softmax,
        exact when bq == 0 -- the general-bias fallback kernel keeps Q/K)
    p = exp(sT/sqrt(C) - 2)  (shift keeps exp < 240 = e4m3 max; softmax is
        shift-invariant since denom uses the same shifted values)   fp8
    v  = h^T wv^T   [HW, C] fp8 (bv folded into boP = bo + wo@bv)
    aT[c, n] = (sum_m v[m,c] p[m,n]) / denom[n]    fp8
    y  = wo @ aT + boP + x                          [C, HW] fp32

All heavy matmuls run fp8e4 with MatmulPerfMode.DoubleRow: operands carry the
contraction split [128 partitions, 2 k-tiles, free], one instruction contracts
256 x 512 output columns (measured ~2.6x fp32r per MAC). Weights are paired +
quantized on host and DMA'd as fp8 (no on-chip conversion).

Engine budget per image (approx): PE 33us, DVE 24us, ACT 21us, GpSimd 15us.
Evacs: Z/V/A on DVE, exp on ACT, OUT on ACT-Copy + gpsimd residual add,
recip via reciprocal_approx_fast, GN affine split gpsimd/DVE.

Sharding: data-parallel over batch; 8 cores x 4 images each.
"""

import math
import os

import numpy as np
import ml_dtypes

import concourse.bass as bass
import concourse.tile as tile
from concourse import bacc, mybir
from concourse.bass_utils import run_bass_kernel_spmd

N_CORES = 8
B, C, H, W = 32, 512, 32, 32
HW = H * W                      # 1024 tokens
BL = B // N_CORES               # 4 images per core
NGRP = 32                       # groupnorm groups
GS = C // NGRP                  # 16 channels per group
EPS = 1e-5
P = 128
NT = C // P                     # 4 channel partition-tiles
KP = NT // 2                    # 2 channel k-tile pairs (DoubleRow)
MT = HW // P                    # 8 token partition-tiles
MP = MT // 2                    # 4 token k-tile pairs
FCH = 512                       # output free-dim chunk (one PSUM bank fp32)
NCH = HW // FCH                 # 2 free chunks per 1024
F32 = mybir.dt.float32
F8 = mybir.dt.float8e4
DR = mybir.MatmulPerfMode.DoubleRow
SCALE = 1.0 / math.sqrt(C)
ESHIFT = -2.0                   # exp shift: keeps exp(s) under e4m3 max 240

ACT_EXP = mybir.ActivationFunctionType.Exp
ACT_LN = mybir.ActivationFunctionType.Ln
ACT_COPY = mybir.ActivationFunctionType.Copy
OP_ADD = mybir.AluOpType.add
OP_MULT = mybir.AluOpType.mult

LAST_EXEC_NS = None
_CACHED = {}


def _build_nc(fused):
    from contextlib import ExitStack

    nc = bacc.Bacc("TRN2", target_bir_lowering=False, debug=False)

    x_d = nc.dram_tensor("x", [BL, C, HW], F32, kind="ExternalInput").ap()
    # paired fp8 weights: [p, k, i, o] = w.T[(2k+i)*128+p, o]
    # fused mode: the wq8 slot carries G^T pairs for Z = G h (G = wk^T wq)
    wq8_d = nc.dram_tensor("wq8", [P, KP, 2, C], F8, kind="ExternalInput").ap()
    wk8_d = nc.dram_tensor("wk8", [P, KP, 2, C], F8, kind="ExternalInput").ap()
    wv8_d = nc.dram_tensor("wv8", [P, KP, 2, C], F8, kind="ExternalInput").ap()
    wo8_d = nc.dram_tensor("wo8", [P, KP, 2, C], F8, kind="ExternalInput").ap()
    ones8_d = nc.dram_tensor("ones8", [P, 2, P], F8, kind="ExternalInput").ap()
    bq_d = nc.dram_tensor("bq", [C], F32, kind="ExternalInput").ap()
    bk_d = nc.dram_tensor("bk", [C], F32, kind="ExternalInput").ap()
    boP_d = nc.dram_tensor("boP", [C], F32, kind="ExternalInput").ap()
    gw_d = nc.dram_tensor("gw", [C], F32, kind="ExternalInput").ap()
    gb_d = nc.dram_tensor("gb", [C], F32, kind="ExternalInput").ap()
    gm_d = nc.dram_tensor("gm", [P, NT, NGRP], F32, kind="ExternalInput").ap()
    gmt_d = nc.dram_tensor("gmt", [NGRP, NT, P], F32, kind="ExternalInput").ap()
    y_d = nc.dram_tensor("y", [BL, C, HW], F32, kind="ExternalOutput").ap()

    x_r = x_d.rearrange("b (t p) n -> b t p n", p=P)
    y_r = y_d.rearrange("b (t p) n -> b t p n", p=P)

    ib = lambda k, d: int(os.environ.get(k, d))  # buf-count knobs for tuning
    with tile.TileContext(nc) as tc, ExitStack() as ctx:
        pool = lambda name, bufs, space="SBUF": ctx.enter_context(
            tc.tile_pool(name=name, bufs=bufs, space=space)
        )
        p_const = pool("const", 1)
        p_x = pool("x", ib("BUF_X", 16))
        p_X = pool("X", ib("BUF_XN", 2))
        p_z = pool("z", 2)
        p_kt = pool("kt", 2)
        p_v = pool("v", 2)
        p_e = pool("exp", ib("BUF_EXP", 8))
        p_a = pool("a", 2)
        p_recip = pool("recip", 2)
        p_out = pool("out", ib("BUF_OUT", 4))
        p_small = pool("small", 4)
        psum = pool("psum", ib("BUF_PSUM", 8), space="PSUM")

        def ps_tile(name, parts=P, free=FCH):
            return psum.tile([parts, free], F32, tag="u", name=name)

        # ---- x loads; image 0 uses finer chunks so GN stats start sooner ----
        def emit_x(b, chunks=NCH):
            xt = []
            fch = HW // chunks
            for t in range(NT):
                xtile = p_x.tile([P, HW], F32, tag="x", name=f"x_{b}_{t}")
                for i in range(chunks):
                    nc.sync.dma_start(
                        out=xtile[:, i * fch : (i + 1) * fch],
                        in_=x_r[b, t][:, i * fch : (i + 1) * fch],
                    )
                xt.append(xtile)
            return xt

        xt0 = emit_x(0, chunks=4)

        # ---- small constants ----
        def load_cols(dram, tag):
            t = p_const.tile([P, NT], F32, tag=tag)
            nc.sync.dma_start(out=t[:], in_=dram.rearrange("(t p) -> p t", p=P))
            return t

        bq_sb = load_cols(bq_d, "bq")
        bk_sb = load_cols(bk_d, "bk")
        boP_sb = load_cols(boP_d, "boP")
        gw_sb = load_cols(gw_d, "gw")
        gb_sb = load_cols(gb_d, "gb")

        gm_sb = p_const.tile([P, NT, NGRP], F32, tag="gm")
        nc.sync.dma_start(out=gm_sb[:], in_=gm_d)
        gmt_sb = p_const.tile([NGRP, NT, P], F32, tag="gmt")
        nc.sync.dma_start(out=gmt_sb[:], in_=gmt_d)
        eps_sb = p_const.tile([P, 1], F32, tag="eps")
        nc.vector.memset(eps_sb[:], EPS)
        esh_sb = p_const.tile([P, 1], F32, tag="esh")
        nc.vector.memset(esh_sb[:], ESHIFT)

        # ---- fp8 weights: direct DMA, no staging/conversion ----
        def load_w8(dram, tag):
            t = p_const.tile([P, KP, 2, C], F8, tag=tag)
            nc.sync.dma_start(out=t[:], in_=dram)
            return t

        wq8 = load_w8(wq8_d, "wq8")          # Z weights (G pairs) when fused
        wk8 = None if fused else load_w8(wk8_d, "wk8")
        wv8 = load_w8(wv8_d, "wv8")
        wo8 = load_w8(wo8_d, "wo8")
        ones8 = p_const.tile([P, 2, P], F8, tag="ones8")
        nc.sync.dma_start(out=ones8[:], in_=ones8_d)

        def emit_gn_stats(b, xt, chunks=NCH):
            """DVE-only per-tile stats: stat2 = [mean, var + mean^2] per channel."""
            fch = HW // chunks
            stat2s = []
            for t in range(NT):
                st = p_small.tile([P, chunks, 6], F32, tag=f"bnst{chunks}")
                for i in range(chunks):
                    nc.vector.bn_stats(
                        out=st[:, i, :], in_=xt[t][:, i * fch : (i + 1) * fch]
                    )
                mv = p_small.tile([P, 2], F32, tag="bnmv")
                nc.vector.bn_aggr(out=mv[:], in_=st[:])
                stat2 = p_small.tile([P, 2], F32, tag="stat2", name=f"stat2_{b}_{t}")
                nc.vector.tensor_copy(out=stat2[:, 0:1], in_=mv[:, 0:1])
                m2 = p_small.tile([P, 1], F32, tag="m2")
                nc.vector.tensor_mul(m2[:], mv[:, 0:1], mv[:, 0:1])
                nc.vector.tensor_add(stat2[:, 1:2], mv[:, 1:2], m2[:])
                stat2s.append(stat2)
            return xt, stat2s

        def emit_gn_reduce(b, state):
            """Group-reduce via PE; rstd = rsqrt(var+eps)."""
            xt, stat2s = state
            psg = ps_tile(f"psg_{b}", parts=NGRP, free=2)
            for t in range(NT):
                nc.tensor.matmul(
                    psg[:], gm_sb[:, t, :], stat2s[t][:],
                    start=(t == 0), stop=(t == NT - 1),
                )
            # gmr: [32 groups, (mean, rstd)]
            gmr = p_small.tile([NGRP, 2], F32, tag="gmr")
            nc.vector.tensor_scalar_mul(gmr[:, 0:1], psg[:, 0:1], 1.0 / GS)
            e2g = p_small.tile([NGRP, 1], F32, tag="e2g")
            nc.vector.tensor_scalar_mul(e2g[:], psg[:, 1:2], 1.0 / GS)
            m2g = p_small.tile([NGRP, 1], F32, tag="m2g")
            nc.vector.tensor_mul(m2g[:], gmr[:, 0:1], gmr[:, 0:1])
            varg = p_small.tile([NGRP, 1], F32, tag="varg")
            nc.vector.tensor_sub(varg[:], e2g[:], m2g[:])
            if fused:
                # DVE-only rsqrt: group var is ~1 for standardized x (host
                # checks and falls back otherwise), so Newton from y0=1
                # converges quadratically; 3 iterations reach <1e-6 for
                # var in [0.5, 2]. Avoids ACT act-table loads entirely.
                ve = p_small.tile([NGRP, 1], F32, tag="ve")
                nc.vector.tensor_scalar_add(ve[:], varg[:], EPS)
                y = p_small.tile([NGRP, 1], F32, tag="nwy")
                nc.vector.tensor_scalar(
                    out=y[:], in0=ve[:], scalar1=-0.5, scalar2=1.5,
                    op0=OP_MULT, op1=OP_ADD,
                )
                for _ in range(2):
                    t1 = p_small.tile([NGRP, 1], F32, tag="nwt")
                    nc.vector.tensor_mul(t1[:], ve[:], y[:])
                    nc.vector.tensor_mul(t1[:], t1[:], y[:])
                    nc.vector.tensor_scalar(
                        out=t1[:], in0=t1[:], scalar1=-0.5, scalar2=1.5,
                        op0=OP_MULT, op1=OP_ADD,
                    )
                    nc.vector.tensor_mul(y[:], y[:], t1[:])
                nc.vector.tensor_copy(out=gmr[:, 1:2], in_=y[:])
            else:
                lng = p_small.tile([NGRP, 1], F32, tag="lng")
                nc.scalar.activation(
                    out=lng[:], in_=varg[:], func=ACT_LN, bias=eps_sb[0:NGRP, :]
                )
                nc.scalar.activation(
                    out=gmr[:, 1:2], in_=lng[:], func=ACT_EXP, scale=-0.5
                )
            return xt, gmr

        def emit_gn_norm(b, state):
            """Broadcast group stats to channels, apply affine -> fp8 pairs.
            Affine split across gpsimd and DVE to halve its latency."""
            xt, gmr = state
            X8 = p_X.tile([P, KP, 2, HW], F8, tag="X", name=f"X_{b}")
            for t in range(NT):
                psb = ps_tile(f"psb_{b}_{t}", free=2)
                nc.tensor.matmul(psb[:], gmt_sb[:, t, :], gmr[:], start=True, stop=True)
                acol = p_small.tile([P, 1], F32, tag="acol")
                nc.vector.tensor_mul(acol[:], psb[:, 1:2], gw_sb[:, t : t + 1])
                tmb = p_small.tile([P, 1], F32, tag="tmb")
                nc.vector.tensor_mul(tmb[:], psb[:, 0:1], acol[:])
                bcol = p_small.tile([P, 1], F32, tag="bcol")
                nc.vector.tensor_sub(bcol[:], gb_sb[:, t : t + 1], tmb[:])
                eng = nc.gpsimd if t < 2 else nc.vector
                eng.tensor_scalar(
                    out=X8[:, t // 2, t % 2, :], in0=xt[t][:],
                    scalar1=acol[:], scalar2=bcol[:], op0=OP_MULT, op1=OP_ADD,
                )
            return xt, X8

        xt_pre = {1: emit_x(1)} if BL > 1 else {}
        gn_state = emit_gn_norm(0, emit_gn_reduce(0, emit_gn_stats(0, xt0, chunks=4)))

        # ---- per-image heavy phases ----
        for b in range(BL):
            xt, X8 = gn_state
            # prefetch x two images ahead so bn_stats(b+1) never waits on DMA
            if b + 2 < BL:
                xt_pre[b + 2] = emit_x(b + 2)
            xt_next = xt_pre.get(b + 1)

            # channel-major projection [C, HW] with DVE evac (+ optional bias)
            def proj_cm(w8, bias_sb, out_pool, bname):
                dst = out_pool.tile(
                    [P, KP, 2, HW], F8, tag=bname, name=f"{bname}_{b}"
                )
                for ot in range(NT):
                    for nch in range(NCH):
                        ps = ps_tile(f"ps_{bname}_{b}_{ot}_{nch}")
                        for k in range(KP):
                            nc.tensor.matmul(
                                ps[:],
                                w8[:, k, :, ot * P : (ot + 1) * P],
                                X8[:, k, :, nch * FCH : (nch + 1) * FCH],
                                start=(k == 0),
                                stop=(k == KP - 1),
                                perf_mode=DR,
                            )
                        dslice = dst[:, ot // 2, ot % 2, nch * FCH : (nch + 1) * FCH]
                        if bias_sb is None:
                            nc.vector.tensor_copy(out=dslice, in_=ps[:])
                        else:
                            nc.vector.tensor_scalar(
                                out=dslice, in0=ps[:],
                                scalar1=bias_sb[:, ot : ot + 1], scalar2=None,
                                op0=OP_ADD,
                            )
                return dst

            if fused:
                Z8 = proj_cm(wq8, None, p_z, "z")
                s_stat, s_mov = X8, Z8
            else:
                QT8 = proj_cm(wq8, bq_sb, p_z, "q")
                KT8 = proj_cm(wk8, bk_sb, p_kt, "k")
                s_stat, s_mov = KT8, QT8

            # scores S^T[m, n] -> exp (fp8, shifted); per-m-pair tiles so PV
            # can start before the whole phase has evacuated
            E8 = [
                p_e.tile([P, 2, HW], F8, tag="exp", name=f"e_{b}_{j}")
                for j in range(MP)
            ]
            for mt in range(MT):
                for nch in range(NCH):
                    psS = ps_tile(f"ps_s_{b}_{mt}_{nch}")
                    for k in range(KP):
                        nc.tensor.matmul(
                            psS[:],
                            s_stat[:, k, :, mt * P : (mt + 1) * P],
                            s_mov[:, k, :, nch * FCH : (nch + 1) * FCH],
                            start=(k == 0),
                            stop=(k == KP - 1),
                            perf_mode=DR,
                        )
                    nc.scalar.activation(
                        out=E8[mt // 2][:, mt % 2, nch * FCH : (nch + 1) * FCH],
                        in_=psS[:], func=ACT_EXP, scale=SCALE, bias=esh_sb[:],
                    )

            # GN(b+1): stats in the S-phase shadow; the affine (gpsimd/DVE)
            # runs during V+PV so X8(b+1) is ready before the next Z-proj
            if xt_next is not None:
                gn_state = emit_gn_norm(
                    b + 1, emit_gn_reduce(b + 1, emit_gn_stats(b + 1, xt_next))
                )

            # V projection (token-major pairs); emitted after S so the PE
            # stays busy while ACT drains the exp backlog PV depends on
            V8 = p_v.tile([P, MP, 2, C], F8, tag="v", name=f"v_{b}")
            for mt in range(MT):
                ps = ps_tile(f"ps_v_{b}_{mt}")
                for k in range(KP):
                    nc.tensor.matmul(
                        ps[:],
                        X8[:, k, :, mt * P : (mt + 1) * P],
                        wv8[:, k, :, :],
                        start=(k == 0),
                        stop=(k == KP - 1),
                        perf_mode=DR,
                    )
                nc.vector.tensor_copy(out=V8[:, mt // 2, mt % 2, :], in_=ps[:])

            # A^T[c, n] accumulated over m-pairs, normalized by 1/denom.
            # colsum emitted after PV c2=0 so the PE never stalls on ACT.
            recip = p_recip.tile([P, HW], F32, tag="recip", name=f"recip_{b}")
            A8 = p_a.tile([P, KP, 2, HW], F8, tag="a", name=f"a_{b}")
            for c2 in range(NT):
                psA = []
                for nch in range(NCH):
                    ps_at = ps_tile(f"ps_a_{b}_{c2}_{nch}")
                    for j in range(MP):
                        nc.tensor.matmul(
                            ps_at[:],
                            V8[:, j, :, c2 * P : (c2 + 1) * P],
                            E8[j][:, :, nch * FCH : (nch + 1) * FCH],
                            start=(j == 0),
                            stop=(j == MP - 1),
                            perf_mode=DR,
                        )
                    psA.append(ps_at)
                if c2 == 0:
                    for nch in range(NCH):
                        psc_t = ps_tile(f"psc_{b}_{nch}")
                        for j in range(MP):
                            nc.tensor.matmul(
                                psc_t[:],
                                ones8[:],
                                E8[j][:, :, nch * FCH : (nch + 1) * FCH],
                                start=(j == 0),
                                stop=(j == MP - 1),
                                perf_mode=DR,
                            )
                        nc.vector.reciprocal_approx_fast(
                            out=recip[:, nch * FCH : (nch + 1) * FCH], in_=psc_t[:]
                        )
                for nch in range(NCH):
                    nc.vector.tensor_mul(
                        A8[:, c2 // 2, c2 % 2, nch * FCH : (nch + 1) * FCH],
                        psA[nch][:],
                        recip[:, nch * FCH : (nch + 1) * FCH],
                    )

            # output projection + bias + residual (fp32 out)
            for co in range(NT):
                for nch in range(NCH):
                    ps = ps_tile(f"ps_o_{b}_{co}_{nch}")
                    for k in range(KP):
                        nc.tensor.matmul(
                            ps[:],
                            wo8[:, k, :, co * P : (co + 1) * P],
                            A8[:, k, :, nch * FCH : (nch + 1) * FCH],
                            start=(k == 0),
                            stop=(k == KP - 1),
                            perf_mode=DR,
                        )
                    ot = p_out.tile([P, FCH], F32, tag="out", name=f"o_{b}_{co}_{nch}")
                    if fused:
                        # boP == 0 here: ACT Copy evac, residual add on gpsimd
                        tmp = p_out.tile(
                            [P, FCH], F32, tag="tmp", name=f"t_{b}_{co}_{nch}"
                        )
                        nc.scalar.activation(
                            out=tmp[:], in_=ps[:], func=ACT_COPY, bias=0.0
                        )
                        nc.gpsimd.tensor_add(
                            ot[:], tmp[:], xt[co][:, nch * FCH : (nch + 1) * FCH]
                        )
                    else:
                        nc.vector.scalar_tensor_tensor(
                            out=ot[:], in0=ps[:], scalar=boP_sb[:, co : co + 1],
                            in1=xt[co][:, nch * FCH : (nch + 1) * FCH],
                            op0=OP_ADD, op1=OP_ADD,
                        )
                    nc.sync.dma_start(
                        out=y_r[b, co][:, nch * FCH : (nch + 1) * FCH],
                        in_=ot[:],
                    )

    nc.compile()
    return nc


def _host_inputs(fused, x, gn_scale, gn_bias, wq, bq, wk, bk, wv, bv, wo, bo):
    f = lambda a: np.ascontiguousarray(np.asarray(a, dtype=np.float32))
    x = f(x).reshape(B, C, HW)
    boP = f(bo) + f(wo) @ f(bv)

    def pair8(wT):
        # [p, k, i, o] = wT[(2k+i)*128+p, o], quantized to e4m3
        t = np.ascontiguousarray(wT).reshape(KP, 2, P, C).transpose(2, 0, 1, 3)
        return np.ascontiguousarray(
            np.clip(t, -240, 240).astype(ml_dtypes.float8_e4m3)
        )

    if fused:
        G = f(wk).T @ f(wq)          # S^T[m,n] = h_m^T G h_n
        wq8 = pair8(G.T)             # stationary pairs of G: [p,k,i,o]=G[o,c]
        wk8 = np.zeros((P, KP, 2, C), ml_dtypes.float8_e4m3)
    else:
        wq8 = pair8(f(wq).T)
        wk8 = pair8(f(wk).T)

    gm = np.zeros((P, NT, NGRP), np.float32)
    gmt = np.zeros((NGRP, NT, P), np.float32)
    for t in range(NT):
        for p in range(P):
            g = (t * P + p) // GS
            gm[p, t, g] = 1.0
            gmt[g, t, p] = 1.0
    ones8 = np.ones((P, 2, P), ml_dtypes.float8_e4m3)

    shared = {
        "wq8": wq8,
        "wk8": wk8,
        "wv8": pair8(f(wv).T),
        "wo8": pair8(f(wo).T),
        "bq": f(bq), "bk": f(bk), "boP": boP,
        "gw": f(gn_scale), "gb": f(gn_bias),
        "gm": gm, "gmt": gmt, "ones8": ones8,
    }
    in_maps = []
    for i in range(N_CORES):
        m = dict(shared)
        m["x"] = np.ascontiguousarray(x[i * BL : (i + 1) * BL])
        in_maps.append(m)
    return in_maps


def kernel(x, gn_scale, gn_bias, wq, bq, wk, bk, wv, bv, wo, bo):
    global LAST_EXEC_NS
    assert x.shape == (B, C, H, W)
    boP = np.asarray(bo, np.float32) + np.asarray(wo, np.float32) @ np.asarray(
        bv, np.float32
    )
    # Z-fusion drops bq into softmax-invariant terms and needs boP == 0 for
    # the bias-free OUT evac; other biases fold exactly in both modes. The
    # fused GN rstd uses Newton-from-1 which needs roughly unit group
    # variance, so oddly-scaled x also routes to the general kernel.
    xs = np.asarray(x, np.float32).ravel()[::1031][:8192]
    fused = (
        not (np.any(np.asarray(bq)) or np.any(boP))
        and 0.5 < float(xs.var()) < 2.0
    )
    if fused not in _CACHED:
        _CACHED[fused] = _build_nc(fused)
    in_maps = _host_inputs(
        fused, x, gn_scale, gn_bias, wq, bq, wk, bk, wv, bv, wo, bo
    )
    trace = os.environ.get("ATT_TRACE", "0") == "1"
    if not trace:
        # the NTFF trace path needs antenv.axon_hooks (shimmed only by our
        # test harness); make sure a stray BASS_TRACE can't drag us into it
        os.environ["BASS_NEVER_TRACE"] = "1"
    else:
        os.environ.pop("BASS_NEVER_TRACE", None)
    kwargs = {}
    tdir = os.environ.get("ATT_TRACE_DIR")
    if tdir:
        kwargs["tmpdir"] = tdir
    res = run_bass_kernel_spmd(
        _CACHED[fused], in_maps, core_ids=list(range(N_CORES)), trace=trace, **kwargs
    )
    LAST_EXEC_NS = res.exec_time_ns
    y = np.concatenate([res.results[i]["y"] for i in range(N_CORES)], axis=0)
    return y.reshape(B, C, H, W).astype(np.float32)


# revision 11
# speedup vs baseline: 1.2349x; 1.0158x over previous
"""Trainium2 Bass kernel: GroupNorm(32) + single-head self-attention block + residual.

fp8 (e4m3) DoubleRow edition with fused score projection. Per image:
    h  = group_norm(x)  (fp32 stats; normalized output quantized to fp8)
    sT[m, n] = h_m^T G h_n,  G = wk^T wq   (one Z = G h projection replaces
        separate Q and K projections; the per-n bias terms cancel in softmax,
        exact when bq == 0 -- the general-bias fallback kernel keeps Q/K)
    p = exp(sT/sqrt(C) - 2)  (shift keeps exp < 240 = e4m3 max; softmax is
        shift-invariant since denom uses the same shifted values)   fp8
    v  = h^T wv^T   [HW, C] fp8 (bv folded into boP = bo + wo@bv)
    aT[c, n] = (sum_m v[m,c] p[m,n]) / denom[n]    fp8
    y  = wo @ aT + boP + x                          [C, HW] fp32

All heavy matmuls run fp8e4 with MatmulPerfMode.DoubleRow: operands carry the
contraction split [128 partitions, 2 k-tiles, free], one instruction contracts
256 x 512 output columns (measured ~2.6x fp32r per MAC). Weights are paired +
quantized on host and DMA'd as fp8 (no on-chip conversion).

Engine budget per image (approx): PE 33us, DVE 24us, ACT 21us, GpSimd 15us.
Evacs: Z/V/A on DVE, exp on ACT, OUT on ACT-Copy + gpsimd residual add,
recip via reciprocal_approx_fast, GN affine split gpsimd/DVE.

Sharding: data-parallel over batch; 8 cores x 4 images each.
"""

import math
import os

import numpy as np
import ml_dtypes

import concourse.bass as bass
import concourse.tile as tile
from concourse import bacc, mybir
from concourse.bass_utils import run_bass_kernel_spmd

N_CORES = 8
B, C, H, W = 32, 512, 32, 32
HW = H * W                      # 1024 tokens
BL = B // N_CORES               # 4 images per core
NGRP = 32                       # groupnorm groups
GS = C // NGRP                  # 16 channels per group
EPS = 1e-5
P = 128
NT = C // P                     # 4 channel partition-tiles
KP = NT // 2                    # 2 channel k-tile pairs (DoubleRow)
MT = HW // P                    # 8 token partition-tiles
MP = MT // 2                    # 4 token k-tile pairs
FCH = 512                       # output free-dim chunk (one PSUM bank fp32)
NCH = HW // FCH                 # 2 free chunks per 1024
F32 = mybir.dt.float32
F8 = mybir.dt.float8e4
DR = mybir.MatmulPerfMode.DoubleRow
SCALE = 1.0 / math.sqrt(C)
ESHIFT = -2.0                   # exp shift: keeps exp(s) under e4m3 max 240

ACT_EXP = mybir.ActivationFunctionType.Exp
ACT_LN = mybir.ActivationFunctionType.Ln
ACT_COPY = mybir.ActivationFunctionType.Copy
OP_ADD = mybir.AluOpType.add
OP_MULT = mybir.AluOpType.mult

LAST_EXEC_NS = None
_CACHED = {}


def _build_nc(fused):
    from contextlib import ExitStack

    nc = bacc.Bacc("TRN2", target_bir_lowering=False, debug=False)

    x_d = nc.dram_tensor("x", [BL, C, HW], F32, kind="ExternalInput").ap()
    # paired fp8 weights: [p, k, i, o] = w.T[(2k+i)*128+p, o]
    # fused mode: the wq8 slot carries G^T pairs for Z = G h (G = wk^T wq)
    wq8_d = nc.dram_tensor("wq8", [P, KP, 2, C], F8, kind="ExternalInput").ap()
    wk8_d = nc.dram_tensor("wk8", [P, KP, 2, C], F8, kind="ExternalInput").ap()
    wv8_d = nc.dram_tensor("wv8", [P, KP, 2, C], F8, kind="ExternalInput").ap()
    wo8_d = nc.dram_tensor("wo8", [P, KP, 2, C], F8, kind="ExternalInput").ap()
    ones8_d = nc.dram_tensor("ones8", [P, 2, P], F8, kind="ExternalInput").ap()
    bq_d = nc.dram_tensor("bq", [C], F32, kind="ExternalInput").ap()
    bk_d = nc.dram_tensor("bk", [C], F32, kind="ExternalInput").ap()
    boP_d = nc.dram_tensor("boP", [C], F32, kind="ExternalInput").ap()
    gw_d = nc.dram_tensor("gw", [C], F32, kind="ExternalInput").ap()
    gb_d = nc.dram_tensor("gb", [C], F32, kind="ExternalInput").ap()
    gm_d = nc.dram_tensor("gm", [P, NT, NGRP], F32, kind="ExternalInput").ap()
    gmt_d = nc.dram_tensor("gmt", [NGRP, NT, P], F32, kind="ExternalInput").ap()
    y_d = nc.dram_tensor("y", [BL, C, HW], F32, kind="ExternalOutput").ap()

    x_r = x_d.rearrange("b (t p) n -> b t p n", p=P)
    y_r = y_d.rearrange("b (t p) n -> b t p n", p=P)

    ib = lambda k, d: int(os.environ.get(k, d))  # buf-count knobs for tuning
    with tile.TileContext(nc) as tc, ExitStack() as ctx:
        pool = lambda name, bufs, space="SBUF": ctx.enter_context(
            tc.tile_pool(name=name, bufs=bufs, space=space)
        )
        p_const = pool("const", 1)
        p_x = pool("x", ib("BUF_X", 16))
        p_X = pool("X", ib("BUF_XN", 2))
        p_z = pool("z", 2)
        p_kt = pool("kt", 2)
        p_v = pool("v", 2)
        p_e = pool("exp", ib("BUF_EXP", 8))
        p_a = pool("a", 2)
        p_recip = pool("recip", 2)
        p_out = pool("out", ib("BUF_OUT", 4))
        p_small = pool("small", 4)
        psum = pool("psum", ib("BUF_PSUM", 8), space="PSUM")

        def ps_tile(name, parts=P, free=FCH):
            return psum.tile([parts, free], F32, tag="u", name=name)

        # ---- x loads: one whole-tile DMA each (4KB contiguous rows) ----
        def emit_x(b):
            xt = []
            for t in range(NT):
                xtile = p_x.tile([P, HW], F32, tag="x", name=f"x_{b}_{t}")
                nc.sync.dma_start(out=xtile[:], in_=x_r[b, t])
                xt.append(xtile)
            return xt

        xt0 = emit_x(0)

        # ---- small constants ----
        def load_cols(dram, tag):
            t = p_const.tile([P, NT], F32, tag=tag)
            nc.sync.dma_start(out=t[:], in_=dram.rearrange("(t p) -> p t", p=P))
            return t

        bq_sb = load_cols(bq_d, "bq")
        bk_sb = load_cols(bk_d, "bk")
        boP_sb = load_cols(boP_d, "boP")
        gw_sb = load_cols(gw_d, "gw")
        gb_sb = load_cols(gb_d, "gb")

        gm_sb = p_const.tile([P, NT, NGRP], F32, tag="gm")
        nc.sync.dma_start(out=gm_sb[:], in_=gm_d)
        gmt_sb = p_const.tile([NGRP, NT, P], F32, tag="gmt")
        nc.sync.dma_start(out=gmt_sb[:], in_=gmt_d)
        eps_sb = p_const.tile([P, 1], F32, tag="eps")
        nc.vector.memset(eps_sb[:], EPS)
        esh_sb = p_const.tile([P, 1], F32, tag="esh")
        nc.vector.memset(esh_sb[:], ESHIFT)

        # ---- fp8 weights: direct DMA, no staging/conversion ----
        def load_w8(dram, tag):
            t = p_const.tile([P, KP, 2, C], F8, tag=tag)
            nc.sync.dma_start(out=t[:], in_=dram)
            return t

        wq8 = load_w8(wq8_d, "wq8")          # Z weights (G pairs) when fused
        wk8 = None if fused else load_w8(wk8_d, "wk8")
        wv8 = load_w8(wv8_d, "wv8")
        wo8 = load_w8(wo8_d, "wo8")
        ones8 = p_const.tile([P, 2, P], F8, tag="ones8")
        nc.sync.dma_start(out=ones8[:], in_=ones8_d)

        def emit_gn_stats(b, xt, chunks=NCH):
            """DVE-only per-tile stats: stat2 = [mean, var + mean^2] per channel."""
            fch = HW // chunks
            stat2s = []
            for t in range(NT):
                st = p_small.tile([P, chunks, 6], F32, tag=f"bnst{chunks}")
                for i in range(chunks):
                    nc.vector.bn_stats(
                        out=st[:, i, :], in_=xt[t][:, i * fch : (i + 1) * fch]
                    )
                mv = p_small.tile([P, 2], F32, tag="bnmv")
                nc.vector.bn_aggr(out=mv[:], in_=st[:])
                stat2 = p_small.tile([P, 2], F32, tag="stat2", name=f"stat2_{b}_{t}")
                nc.vector.tensor_copy(out=stat2[:, 0:1], in_=mv[:, 0:1])
                m2 = p_small.tile([P, 1], F32, tag="m2")
                nc.vector.tensor_mul(m2[:], mv[:, 0:1], mv[:, 0:1])
                nc.vector.tensor_add(stat2[:, 1:2], mv[:, 1:2], m2[:])
                stat2s.append(stat2)
            return xt, stat2s

        def emit_gn_reduce(b, state):
            """Group-reduce via PE; rstd = rsqrt(var+eps)."""
            xt, stat2s = state
            psg = ps_tile(f"psg_{b}", parts=NGRP, free=2)
            for t in range(NT):
                nc.tensor.matmul(
                    psg[:], gm_sb[:, t, :], stat2s[t][:],
                    start=(t == 0), stop=(t == NT - 1),
                )
            # gmr: [32 groups, (mean, rstd)]
            gmr = p_small.tile([NGRP, 2], F32, tag="gmr")
            nc.vector.tensor_scalar_mul(gmr[:, 0:1], psg[:, 0:1], 1.0 / GS)
            e2g = p_small.tile([NGRP, 1], F32, tag="e2g")
            nc.vector.tensor_scalar_mul(e2g[:], psg[:, 1:2], 1.0 / GS)
            m2g = p_small.tile([NGRP, 1], F32, tag="m2g")
            nc.vector.tensor_mul(m2g[:], gmr[:, 0:1], gmr[:, 0:1])
            varg = p_small.tile([NGRP, 1], F32, tag="varg")
            nc.vector.tensor_sub(varg[:], e2g[:], m2g[:])
            if fused:
                # DVE-only rsqrt: group var is ~1 for standardized x (host
                # checks and falls back otherwise), so Newton from y0=1
                # converges quadratically; 3 iterations reach <1e-6 for
                # var in [0.5, 2]. Avoids ACT act-table loads entirely.
                ve = p_small.tile([NGRP, 1], F32, tag="ve")
                nc.vector.tensor_scalar_add(ve[:], varg[:], EPS)
                y = p_small.tile([NGRP, 1], F32, tag="nwy")
                nc.vector.tensor_scalar(
                    out=y[:], in0=ve[:], scalar1=-0.5, scalar2=1.5,
                    op0=OP_MULT, op1=OP_ADD,
                )
                for _ in range(2):
                    t1 = p_small.tile([NGRP, 1], F32, tag="nwt")
                    nc.vector.tensor_mul(t1[:], ve[:], y[:])
                    nc.vector.tensor_mul(t1[:], t1[:], y[:])
                    nc.vector.tensor_scalar(
                        out=t1[:], in0=t1[:], scalar1=-0.5, scalar2=1.5,
                        op0=OP_MULT, op1=OP_ADD,
                    )
                    nc.vector.tensor_mul(y[:], y[:], t1[:])
                nc.vector.tensor_copy(out=gmr[:, 1:2], in_=y[:])
            else:
                lng = p_small.tile([NGRP, 1], F32, tag="lng")
                nc.scalar.activation(
                    out=lng[:], in_=varg[:], func=ACT_LN, bias=eps_sb[0:NGRP, :]
                )
                nc.scalar.activation(
                    out=gmr[:, 1:2], in_=lng[:], func=ACT_EXP, scale=-0.5
                )
            return xt, gmr

        def emit_gn_norm(b, state):
            """Broadcast group stats to channels, apply affine -> fp8 pairs.
            Affine split across gpsimd and DVE to halve its latency."""
            xt, gmr = state
            X8 = p_X.tile([P, KP, 2, HW], F8, tag="X", name=f"X_{b}")
            for t in range(NT):
                psb = ps_tile(f"psb_{b}_{t}", free=2)
                nc.tensor.matmul(psb[:], gmt_sb[:, t, :], gmr[:], start=True, stop=True)
                acol = p_small.tile([P, 1], F32, tag="acol")
                nc.vector.tensor_mul(acol[:], psb[:, 1:2], gw_sb[:, t : t + 1])
                tmb = p_small.tile([P, 1], F32, tag="tmb")
                nc.vector.tensor_mul(tmb[:], psb[:, 0:1], acol[:])
                bcol = p_small.tile([P, 1], F32, tag="bcol")
                nc.vector.tensor_sub(bcol[:], gb_sb[:, t : t + 1], tmb[:])
                eng = nc.gpsimd if t < 2 else nc.vector
                eng.tensor_scalar(
                    out=X8[:, t // 2, t % 2, :], in0=xt[t][:],
                    scalar1=acol[:], scalar2=bcol[:], op0=OP_MULT, op1=OP_ADD,
                )
            return xt, X8

        xt_pre = {1: emit_x(1)} if BL > 1 else {}
        gn_state = emit_gn_norm(0, emit_gn_reduce(0, emit_gn_stats(0, xt0)))

        # ---- per-image heavy phases ----
        for b in range(BL):
            xt, X8 = gn_state
            # prefetch x two images ahead so bn_stats(b+1) never waits on DMA
            if b + 2 < BL:
                xt_pre[b + 2] = emit_x(b + 2)
            xt_next = xt_pre.get(b + 1)

            # channel-major projection [C, HW] with DVE evac (+ optional bias)
            def proj_cm(w8, bias_sb, out_pool, bname):
                dst = out_pool.tile(
                    [P, KP, 2, HW], F8, tag=bname, name=f"{bname}_{b}"
                )
                for ot in range(NT):
                    for nch in range(NCH):
                        ps = ps_tile(f"ps_{bname}_{b}_{ot}_{nch}")
                        for k in range(KP):
                            nc.tensor.matmul(
                                ps[:],
                                w8[:, k, :, ot * P : (ot + 1) * P],
                                X8[:, k, :, nch * FCH : (nch + 1) * FCH],
                                start=(k == 0),
                                stop=(k == KP - 1),
                                perf_mode=DR,
                            )
                        dslice = dst[:, ot // 2, ot % 2, nch * FCH : (nch + 1) * FCH]
                        if bias_sb is None:
                            nc.scalar.activation(
                                out=dslice, in_=ps[:], func=ACT_COPY, bias=0.0
                            )
                        else:
                            nc.vector.tensor_scalar(
                                out=dslice, in0=ps[:],
                                scalar1=bias_sb[:, ot : ot + 1], scalar2=None,
                                op0=OP_ADD,
                            )
                return dst

            if fused:
                Z8 = proj_cm(wq8, None, p_z, "z")
                s_stat, s_mov = X8, Z8
            else:
                QT8 = proj_cm(wq8, bq_sb, p_z, "q")
                KT8 = proj_cm(wk8, bk_sb, p_kt, "k")
                s_stat, s_mov = KT8, QT8

            # scores S^T[m, n] -> exp (fp8, shifted); per-m-pair tiles so PV
            # can start before the whole phase has evacuated
            E8 = [
                p_e.tile([P, 2, HW], F8, tag="exp", name=f"e_{b}_{j}")
                for j in range(MP)
            ]
            for mt in range(MT):
                for nch in range(NCH):
                    psS = ps_tile(f"ps_s_{b}_{mt}_{nch}")
                    for k in range(KP):
                        nc.tensor.matmul(
                            psS[:],
                            s_stat[:, k, :, mt * P : (mt + 1) * P],
                            s_mov[:, k, :, nch * FCH : (nch + 1) * FCH],
                            start=(k == 0),
                            stop=(k == KP - 1),
                            perf_mode=DR,
                        )
                    nc.scalar.activation(
                        out=E8[mt // 2][:, mt % 2, nch * FCH : (nch + 1) * FCH],
                        in_=psS[:], func=ACT_EXP, scale=SCALE, bias=esh_sb[:],
                    )

            # GN(b+1): stats in the S-phase shadow; the affine (gpsimd/DVE)
            # runs during V+PV so X8(b+1) is ready before the next Z-proj
            if xt_next is not None:
                gn_state = emit_gn_norm(
                    b + 1, emit_gn_reduce(b + 1, emit_gn_stats(b + 1, xt_next))
                )

            # V projection (token-major pairs); emitted after S so the PE
            # stays busy while ACT drains the exp backlog PV depends on
            V8 = p_v.tile([P, MP, 2, C], F8, tag="v", name=f"v_{b}")
            for mt in range(MT):
                ps = ps_tile(f"ps_v_{b}_{mt}")
                for k in range(KP):
                    nc.tensor.matmul(
                        ps[:],
                        X8[:, k, :, mt * P : (mt + 1) * P],
                        wv8[:, k, :, :],
                        start=(k == 0),
                        stop=(k == KP - 1),
                        perf_mode=DR,
                    )
                nc.vector.tensor_copy(out=V8[:, mt // 2, mt % 2, :], in_=ps[:])

            # A^T[c, n] accumulated over m-pairs, normalized by 1/denom.
            # colsum emitted after PV c2=0 so the PE never stalls on ACT.
            recip = p_recip.tile([P, HW], F32, tag="recip", name=f"recip_{b}")
            A8 = p_a.tile([P, KP, 2, HW], F8, tag="a", name=f"a_{b}")
            for c2 in range(NT):
                psA = []
                for nch in range(NCH):
                    ps_at = ps_tile(f"ps_a_{b}_{c2}_{nch}")
                    for j in range(MP):
                        nc.tensor.matmul(
                            ps_at[:],
                            V8[:, j, :, c2 * P : (c2 + 1) * P],
                            E8[j][:, :, nch * FCH : (nch + 1) * FCH],
                            start=(j == 0),
                            stop=(j == MP - 1),
                            perf_mode=DR,
                        )
                    psA.append(ps_at)
                if c2 == 0:
                    for nch in range(NCH):
                        psc_t = ps_tile(f"psc_{b}_{nch}")
                        for j in range(MP):
                            nc.tensor.matmul(
                                psc_t[:],
                                ones8[:],
                                E8[j][:, :, nch * FCH : (nch + 1) * FCH],
                                start=(j == 0),
                                stop=(j == MP - 1),
                                perf_mode=DR,
                            )
                        nc.vector.reciprocal_approx_fast(
                            out=recip[:, nch * FCH : (nch + 1) * FCH], in_=psc_t[:]
                        )
                for nch in range(NCH):
                    nc.vector.tensor_mul(
                        A8[:, c2 // 2, c2 % 2, nch * FCH : (nch + 1) * FCH],
                        psA[nch][:],
                        recip[:, nch * FCH : (nch + 1) * FCH],
                    )

            # output projection + bias + residual (fp32 out)
            for co in range(NT):
                for nch in range(NCH):
                    ps = ps_tile(f"ps_o_{b}_{co}_{nch}")
                    for k in range(KP):
                        nc.tensor.matmul(
                            ps[:],
                            wo8[:, k, :, co * P : (co + 1) * P],
                            A8[:, k, :, nch * FCH : (nch + 1) * FCH],
                            start=(k == 0),
                            stop=(k == KP - 1),
                            perf_mode=DR,
                        )
                    ot = p_out.tile([P, FCH], F32, tag="out", name=f"o_{b}_{co}_{nch}")
                    if fused and b == BL - 1:
                        nc.vector.scalar_tensor_tensor(
                            out=ot[:], in0=ps[:], scalar=0.0,
                            in1=xt[co][:, nch * FCH : (nch + 1) * FCH],
                            op0=OP_ADD, op1=OP_ADD,
                        )
                    elif fused:
                        # boP == 0 here: ACT Copy evac, residual add on gpsimd
                        tmp = p_out.tile(
                            [P, FCH], F32, tag="tmp", name=f"t_{b}_{co}_{nch}"
                        )
                        nc.scalar.activation(
                            out=tmp[:], in_=ps[:], func=ACT_COPY, bias=0.0
                        )
                        nc.gpsimd.tensor_add(
                            ot[:], tmp[:], xt[co][:, nch * FCH : (nch + 1) * FCH]
                        )
                    else:
                        nc.vector.scalar_tensor_tensor(
                            out=ot[:], in0=ps[:], scalar=boP_sb[:, co : co + 1],
                            in1=xt[co][:, nch * FCH : (nch + 1) * FCH],
                            op0=OP_ADD, op1=OP_ADD,
                        )
                    nc.sync.dma_start(
                        out=y_r[b, co][:, nch * FCH : (nch + 1) * FCH],
                        in_=ot[:],
                    )

    nc.compile()
    return nc


def _host_inputs(fused, x, gn_scale, gn_bias, wq, bq, wk, bk, wv, bv, wo, bo):
    f = lambda a: np.ascontiguousarray(np.asarray(a, dtype=np.float32))
    x = f(x).reshape(B, C, HW)
    boP = f(bo) + f(wo) @ f(bv)

    def pair8(wT):
        # [p, k, i, o] = wT[(2k+i)*128+p, o], quantized to e4m3
        t = np.ascontiguousarray(wT).reshape(KP, 2, P, C).transpose(2, 0, 1, 3)
        return np.ascontiguousarray(
            np.clip(t, -240, 240).astype(ml_dtypes.float8_e4m3)
        )

    if fused:
        G = f(wk).T @ f(wq)          # S^T[m,n] = h_m^T G h_n
        wq8 = pair8(G.T)             # stationary pairs of G: [p,k,i,o]=G[o,c]
        wk8 = np.zeros((P, KP, 2, C), ml_dtypes.float8_e4m3)
    else:
        wq8 = pair8(f(wq).T)
        wk8 = pair8(f(wk).T)

    gm = np.zeros((P, NT, NGRP), np.float32)
    gmt = np.zeros((NGRP, NT, P), np.float32)
    for t in range(NT):
        for p in range(P):
            g = (t * P + p) // GS
            gm[p, t, g] = 1.0
            gmt[g, t, p] = 1.0
    ones8 = np.ones((P, 2, P), ml_dtypes.float8_e4m3)

    shared = {
        "wq8": wq8,
        "wk8": wk8,
        "wv8": pair8(f(wv).T),
        "wo8": pair8(f(wo).T),
        "bq": f(bq), "bk": f(bk), "boP": boP,
        "gw": f(gn_scale), "gb": f(gn_bias),
        "gm": gm, "gmt": gmt, "ones8": ones8,
    }
    in_maps = []
    for i in range(N_CORES):
        m = dict(shared)
        m["x"] = np.ascontiguousarray(x[i * BL : (i + 1) * BL])
        in_maps.append(m)
    return in_maps


def kernel(x, gn_scale, gn_bias, wq, bq, wk, bk, wv, bv, wo, bo):
    global LAST_EXEC_NS
    assert x.shape == (B, C, H, W)
    boP = np.asarray(bo, np.float32) + np.asarray(wo, np.float32) @ np.asarray(
        bv, np.float32
    )
    # Z-fusion drops bq into softmax-invariant terms and needs boP == 0 for
    # the bias-free OUT evac; other biases fold exactly in both modes. The
    # fused GN rstd uses Newton-from-1 which needs roughly unit group
    # variance, so oddly-scaled x also routes to the general kernel.
    xs = np.asarray(x, np.float32).ravel()[::1031][:8192]
    fused = (
        not (np.any(np.asarray(bq)) or np.any(boP))
        and 0.5 < float(xs.var()) < 2.0
    )
    if fused not in _CACHED:
        _CACHED[fused] = _build_nc(fused)
    in_maps = _host_inputs(
        fused, x, gn_scale, gn_bias, wq, bq, wk, bk, wv, bv, wo, bo
    )
    trace = os.environ.get("ATT_TRACE", "0") == "1"
    if not trace:
        # the NTFF trace path needs antenv.axon_hooks (shimmed only by our
        # test harness); make sure a stray BASS_TRACE can't drag us into it
        os.environ["BASS_NEVER_TRACE"] = "1"
    else:
        os.environ.pop("BASS_NEVER_TRACE", None)
    kwargs = {}
    tdir = os.environ.get("ATT_TRACE_DIR")
    if tdir:
        kwargs["tmpdir"] = tdir
    res = run_bass_kernel_spmd(
        _CACHED[fused], in_maps, core_ids=list(range(N_CORES)), trace=trace, **kwargs
    )
    LAST_EXEC_NS = res.exec_time_ns
    y = np.concatenate([res.results[i]["y"] for i in range(N_CORES)], axis=0)
    return y.reshape(B, C, H, W).astype(np.float32)
